# revision 5
# baseline (speedup 1.0000x reference)
"""Trainium2 Bass kernel for nn_DWTExtractor: 2-level Haar DWT + bilinear 2x upsample.

Input  x: (32, 1, 1024, 1024) fp32
Output y: (32, 6, 512, 512) fp32 = [cH1, cV1, cD1, cH2u, cV2u, cD2u]

Sharding: pure batch data-parallel, 4 images per core across 8 cores.

v2 design (fused sum/diff + even/odd-split matmuls):
  - fp16 datapath (host converts, ~1e-3 rel err).
  - L1 Haar: ONE weight matrix WF [128,128] whose output partitions are
    [row-pair sums | row-pair diffs]; two matmuls per 128-row block with
    rhs = even / odd W columns (strided rhs AP). PE columns halve vs the
    v1 PS/PD scheme and the PSUM evacuation becomes a contiguous copy.
  - Combines U = A+B = [cA1|cV1], V = A-B = [cH1|cD1] run as all-SBUF
    packed fp16 tensor_tensor (DVE 2x mode), written straight into
    [128, 4096] staging (block-column layout), so each L1 band leaves
    as ONE output DMA.
  - L2 identical trick on cA1 (= Ustg partitions 0..63) with zero-padded
    weights WL2A/WL2B; cV1 junk partitions contract against zeros.
  - cH2/cV2/cD2 repacked (6 SBUF->SBUF DMAs, uniform partition shift
    each - multi-range-partition APs silently drop the shifted half!)
    into b3all [128, 1536] row-major; W-upsample = 2 strided
    scalar_tensor_tensor + 1 edge op over 4-dim views into wall
    [128, 3072] (= w0|w1) - replaces v1's slow GPSIMD path.
  - H-upsample unchanged from v1: 12 matmuls with halo row swap.
  - L2 PSUM accumulation groups must NOT interleave in a bank
    (start=True of one group corrupts the other open group).
  - Triggers: Sync = input/repack/halo (HWDGE), GPSIMD = band outputs
    (SWDGE) + 2 blocks' combines; DVE/ACT keep pure compute.
"""

import numpy as np

import concourse.bass as bass
import concourse.tile as tile
import concourse.mybir as mybir
from concourse import bacc, bass_utils

F32 = mybir.dt.float32
F16 = mybir.dt.float16
AL = mybir.AluOpType

B, H, W = 32, 1024, 1024
NCORES = 8
IMG = B // NCORES  # images per core
HL, WL = H // 2, W // 2  # 512 (level-1 band size)
H2, W2 = H // 4, W // 4  # 256 (level-2 band size)
P = 128


def _build_w16() -> np.ndarray:
    """(128, 7*128) fp16: WF | WL2A | WL2B | U0 | U1p | U2p | U3."""
    wf = np.zeros((P, P), np.float16)
    for i in range(64):
        wf[2 * i, i] = 0.5
        wf[2 * i + 1, i] = 0.5
        wf[2 * i, 64 + i] = 0.5
        wf[2 * i + 1, 64 + i] = -0.5

    wl2a = np.zeros((P, P), np.float16)
    wl2b = np.zeros((P, P), np.float16)
    for i in range(32):
        wl2a[2 * i, i] = 0.5
        wl2a[2 * i + 1, i] = 0.5
        wl2a[2 * i, 64 + i] = 0.5
        wl2a[2 * i + 1, 64 + i] = -0.5
        wl2b[2 * i, 32 + i] = 0.5
        wl2b[2 * i + 1, 32 + i] = 0.5
        wl2b[2 * i, 96 + i] = 0.5
        wl2b[2 * i + 1, 96 + i] = -0.5

    u_full = np.zeros((H2, HL), np.float32)
    for m in range(HL):
        k = m // 2
        taps = [(k, 0.75), (k - 1, 0.25)] if m % 2 == 0 else [(k, 0.75), (k + 1, 0.25)]
        for src, wgt in taps:
            u_full[min(max(src, 0), H2 - 1), m] += wgt
    u_full *= 0.25
    u0 = u_full[0:128, 0:128].astype(np.float16)
    u1p = u_full[0:128, 128:256].astype(np.float16)
    u1p[0, :] = u_full[128, 128:256].astype(np.float16)  # halo tap row
    u2p = u_full[128:256, 256:384].astype(np.float16)
    u2p[127, :] = u_full[127, 256:384].astype(np.float16)  # halo tap row
    u3 = u_full[128:256, 384:512].astype(np.float16)

    return np.concatenate([wf, wl2a, wl2b, u0, u1p, u2p, u3], axis=1)


def build_nc() -> "bacc.Bacc":
    nc = bacc.Bacc(
        "TRN2", target_bir_lowering=False, debug=False, num_devices=NCORES,
        name="dwt_extractor",
    )
    x_d = nc.dram_tensor("xc", [IMG, H, W], F16, kind="ExternalInput")
    w16_d = nc.dram_tensor("w16", [P, 7 * P], F16, kind="ExternalInput")
    y_d = nc.dram_tensor("yc", [IMG, 6, HL, WL], F16, kind="ExternalOutput")

    with tile.TileContext(nc) as tc:
        with (
            tc.tile_pool(name="consts", bufs=1) as cpool,
            tc.tile_pool(name="xin", bufs=3) as xpool,
            tc.tile_pool(name="ab", bufs=3) as abpool,
            tc.tile_pool(name="uv", bufs=2) as uvpool,
            tc.tile_pool(name="l2", bufs=2) as l2pool,
            tc.tile_pool(name="uv2", bufs=2) as uv2pool,
            tc.tile_pool(name="b3", bufs=2) as b3pool,
            tc.tile_pool(name="wtile", bufs=2) as wpool,
            tc.tile_pool(name="stg2", bufs=2) as stpool,
            tc.tile_pool(name="psL1", bufs=2, space="PSUM") as psL1,
            tc.tile_pool(name="psL2", bufs=1, space="PSUM") as psL2,
            tc.tile_pool(name="psUp", bufs=3, space="PSUM") as psUp,
        ):
            w16 = cpool.tile([P, 7 * P], F16)
            nc.sync.dma_start(w16[:], w16_d[:])
            blk = lambda i: w16[:, i * P : (i + 1) * P]
            WF, WL2A, WL2B = blk(0), blk(1), blk(2)
            U0, U1p, U2p, U3 = blk(3), blk(4), blk(5), blk(6)

            def l1_pair(b, up, Ustg, Vstg):
                """Two 128-row blocks (2up, 2up+1): one load, fused S/D
                matmuls on even/odd cols, evac, combine into U/V staging."""
                xu = xpool.tile([P, 2048], F16, tag="x")
                src = x_d[b, 256 * up : 256 * (up + 1), :]
                nc.sync.dma_start(
                    xu[:].rearrange("p (t w) -> p t w", t=2),
                    src.rearrange("(t p) w -> p t w", t=2),
                )
                for t in range(2):
                    u = 2 * up + t
                    xb = xu[:, 1024 * t : 1024 * (t + 1)]
                    ps = psL1.tile([P, 1024], F32, tag="ab")
                    nc.tensor.matmul(ps[:, 0:512], WF, xb[:, 0:1024:2],
                                     start=True, stop=True)
                    nc.tensor.matmul(ps[:, 512:1024], WF, xb[:, 1:1024:2],
                                     start=True, stop=True)
                    ab = abpool.tile([P, 1024], F16, tag="ab")
                    nc.scalar.copy(ab[:], ps[:])
                    o = 512 * u
                    eng = nc.gpsimd if u in (1, 5) else nc.vector
                    eng.tensor_tensor(Ustg[:, o : o + 512],
                                      ab[:, 0:512], ab[:, 512:1024], AL.add)
                    eng.tensor_tensor(Vstg[:, o : o + 512],
                                      ab[:, 0:512], ab[:, 512:1024], AL.subtract)

            def l2_group(g, Ustg, Ustg2, Vstg2):
                """cA1 rows 128g..128g+127 -> [cA2|cV2], [cH2|cD2] at
                cols 256g of stg2 tiles. Groups must not interleave."""
                ps2 = psL2.tile([P, 512], F32, tag="l2")
                ue0 = Ustg[:, 1024 * g : 1024 * g + 512]
                ue1 = Ustg[:, 1024 * g + 512 : 1024 * g + 1024]
                nc.tensor.matmul(ps2[:, 0:256], WL2A, ue0[:, 0:512:2],
                                 start=True, stop=False)
                nc.tensor.matmul(ps2[:, 0:256], WL2B, ue1[:, 0:512:2],
                                 start=False, stop=True)
                nc.tensor.matmul(ps2[:, 256:512], WL2A, ue0[:, 1:512:2],
                                 start=True, stop=False)
                nc.tensor.matmul(ps2[:, 256:512], WL2B, ue1[:, 1:512:2],
                                 start=False, stop=True)
                a2 = l2pool.tile([P, 512], F16, tag="a2b2")
                nc.scalar.copy(a2[:], ps2[:])
                o = 256 * g
                nc.vector.tensor_tensor(Ustg2[:, o : o + 256],
                                        a2[:, 0:256], a2[:, 256:512], AL.add)
                nc.vector.tensor_tensor(Vstg2[:, o : o + 256],
                                        a2[:, 0:256], a2[:, 256:512], AL.subtract)

            def wup_stage(b, Ustg2, Vstg2):
                """Repack stg2 -> b3all row-major, W-upsample into wall
                [128, 3072] (= [w-tile s][band h][interleaved 512])."""
                b3all = b3pool.tile([P, 1536], F16, tag="b3", name="b3all")
                wall = wpool.tile([P, 3072], F16, tag="wall", name="wall")
                dv = b3all[:].rearrange("(q p) (s h c) -> q p s h c", q=2, s=2, h=3)
                # band order H | V | D; sources: H = Vstg2[0:64],
                # V = Ustg2[64:128], D = Vstg2[64:128]
                for bi, (stg2, lo) in enumerate(
                        ((Vstg2, 0), (Ustg2, 64), (Vstg2, 64))):
                    sv = stg2[lo : lo + 64, :].rearrange(
                        "p (s t c) -> p s t c", s=2, t=2)
                    # one trigger per target partition half: uniform shift
                    nc.sync.dma_start(dv[0, :, :, bi, :], sv[:, :, 0, :])
                    nc.sync.dma_start(dv[1, :, :, bi, :], sv[:, :, 1, :])
                s4 = b3all[:].rearrange("p (s h c) -> p s h c", s=2, h=3)
                d4 = wall[:].rearrange("p (s h c) -> p s h c", s=2, h=3)
                nc.vector.scalar_tensor_tensor(
                    d4[:, :, :, 2:511:2], s4[:, :, :, 1:256], 3.0,
                    s4[:, :, :, 0:255], AL.mult, AL.add)
                nc.vector.scalar_tensor_tensor(
                    d4[:, :, :, 1:510:2], s4[:, :, :, 0:255], 3.0,
                    s4[:, :, :, 1:256], AL.mult, AL.add)
                nc.vector.tensor_scalar_mul(
                    d4[:, :, :, 0:512:511], s4[:, :, :, 0:256:255], 4.0)
                return wall

            def evac_up(dst_ap, src_ap, k):
                if k % 3 == 2:
                    nc.vector.tensor_copy(dst_ap, src_ap)
                else:
                    nc.scalar.copy(dst_ap, src_ap)

            def stage_b1(b, wall, sts):
                """H-up blocks 0 and 3 + halo row swaps for image b."""
                k = 0
                for j, Uw, wo in ((0, U0, 0), (3, U3, 1536)):
                    for band in range(3):
                        if j == 0:
                            st = stpool.tile([P, 2048], F16,
                                             tag=f"s2b{band}", name=f"s2b{band}")
                            sts.append(st)
                        else:
                            st = sts[band]
                        up = psUp.tile([P, 512], F32, tag="up")
                        nc.tensor.matmul(
                            up[:], Uw, wall[:, wo + 512 * band : wo + 512 * (band + 1)],
                            start=True, stop=True)
                        evac_up(st[:, 512 * j : 512 * j + 512], up[:], k)
                        k += 1
                # halo: w0 row0 <- w1 row0; w1 row127 <- w0 row127
                nc.sync.dma_start(wall[0:1, 0:1536], wall[0:1, 1536:3072])
                nc.sync.dma_start(wall[127:128, 1536:3072], wall[127:128, 0:1536])

            def stage_b2(b, wall, sts):
                """H-up blocks 1 and 2 (halo'd) + output DMA for image b."""
                k = 3
                for j, Uw, wo in ((1, U1p, 0), (2, U2p, 1536)):
                    for band in range(3):
                        up = psUp.tile([P, 512], F32, tag="up")
                        nc.tensor.matmul(
                            up[:], Uw, wall[:, wo + 512 * band : wo + 512 * (band + 1)],
                            start=True, stop=True)
                        evac_up(sts[band][:, 512 * j : 512 * j + 512], up[:], k)
                        k += 1
                for band in range(3):
                    dst = y_d[b, 3 + band]
                    nc.gpsimd.dma_start(
                        dst.rearrange("(u p) w -> p u w", u=4),
                        sts[band][:].rearrange("p (u w) -> p u w", u=4))

            pending = None
            for b in range(IMG):
                Ustg = uvpool.tile([P, 4096], F16, tag="U", name="Ustg")
                Vstg = uvpool.tile([P, 4096], F16, tag="V", name="Vstg")
                Ustg2 = uv2pool.tile([P, 1024], F16, tag="U2", name="Ustg2")
                Vstg2 = uv2pool.tile([P, 1024], F16, tag="V2", name="Vstg2")
                l1_pair(b, 0, Ustg, Vstg)
                l1_pair(b, 1, Ustg, Vstg)
                if pending is not None:
                    stage_b1(pending[0], pending[1], pending[2])
                l1_pair(b, 2, Ustg, Vstg)
                l1_pair(b, 3, Ustg, Vstg)
                for g in range(4):
                    l2_group(g, Ustg, Ustg2, Vstg2)
                # L1 band outputs: cH1=V[0:64], cV1=U[64:128], cD1=V[64:128]
                for band, (stg, lo) in enumerate(
                        ((Vstg, 0), (Ustg, 64), (Vstg, 64))):
                    src = stg[lo : lo + 64, :].rearrange(
                        "p (u w) -> p u w", u=8)
                    nc.gpsimd.dma_start(
                        y_d[b, band].rearrange("(u p) w -> p u w", u=8), src)
                wall = wup_stage(b, Ustg2, Vstg2)
                if pending is not None:
                    stage_b2(pending[0], pending[1], pending[2])
                pending = (b, wall, [])
            stage_b1(pending[0], pending[1], pending[2])
            stage_b2(pending[0], pending[1], pending[2])

    nc.compile()
    return nc


_NC_CACHE = None
LAST_RESULTS = None


def kernel(**inputs) -> np.ndarray:
    global _NC_CACHE, LAST_RESULTS
    trace = bool(inputs.pop("_trace", False))
    x = np.asarray(inputs["x"])
    assert x.shape == (B, 1, H, W), x.shape
    x16 = np.ascontiguousarray(x.astype(np.float16))
    if _NC_CACHE is None:
        _NC_CACHE = build_nc()
    nc = _NC_CACHE
    w16 = _build_w16()
    in_maps = [
        {"xc": np.ascontiguousarray(x16[IMG * c : IMG * (c + 1), 0]), "w16": w16}
        for c in range(NCORES)
    ]
    res = bass_utils.run_bass_kernel_spmd(
        nc, in_maps, core_ids=list(range(NCORES)), trace=trace
    )
    LAST_RESULTS = res
    out = np.concatenate([res.results[c]["yc"] for c in range(NCORES)], axis=0)
    return out.astype(np.float32)


if __name__ == "__main__":
    rng = np.random.default_rng(0)
    x = rng.standard_normal((B, 1, H, W), dtype=np.float32)
    y = kernel(x=x)
    print("kernel output:", y.shape, y.dtype)
